# revision 3
# baseline (speedup 1.0000x reference)
"""Trainium2 Bass kernel v3 for the AttentionBlock.

Sharding: 8 cores = (batch 0..3) x (half of the 4096 query positions);
the host rotates keys so each core's queries are cols 0:2048.

Structural changes vs v2 (44.1us HW):
  - Scores via fp8 DoubleRow with M=128 output rows: each 128-key j-tile
    is ONE 256-cycle DR matmul (half the fp16 cost).  The contraction is
    a 198-row packed layout [z_hi; z_lo; z_hi] x [u_hi; u_hi; u_lo]
    (e4m3 value+residual) so the fp8 product is z.u - z_lo.u_lo, with
    ~0.1% score error at unchanged PE cost (DR charges per output row).
  - u = C^T x per q-block, where C = (GN/bias-folded Wq) @ (folded Wk)
    [65,66] is built once per rep from host Dmain = [Wq^T;bq] @ Wk and
    the device GN stats.  This removes the whole q/k projection pipeline
    (two matmul streams + PSUM->SBUF copies).  The key-side bias fold is
    dropped entirely: it only adds per-query constants to the scores,
    which softmax cancels.
  - PV+sums fused: w22 has a 65th=SUMSC column fed by the x ones-row, so
    wg tiles are [128,8,66-in-80] (16B-aligned k-tile stride) and one
    DoubleRow matmul per 256 keys yields [65,512] = 64 channels + the
    softmax denominator row.  Eliminates the separate sums matmuls.
  - Bias rows are built with matmuls that output directly at partition
    64 (bs1=[b_s;1] against a 65-partition cpack) -- tensor ops with
    partition-base-mismatched operands corrupt data.
  - PE warmup spins before x arrives to climb the DVFS pstate ramp.

Engine busy (cost model, per core): PE 25us (was 45.6), ACT 57, DVE 48.
Measured marginal per-iteration on HW: 13.0us vs v2's 26.2us (same
paired-differencing harness).
"""

import numpy as np
import ml_dtypes

import concourse.bacc as bacc
import concourse.bass as bass
import concourse.tile as tile
from concourse import mybir
from concourse.bass_utils import run_bass_kernel_spmd

F32 = mybir.dt.float32
F16 = mybir.dt.float16
E4 = mybir.dt.float8e4
E5 = mybir.dt.float8e5
U8 = mybir.dt.uint8
AF = mybir.ActivationFunctionType
ALU = mybir.AluOpType
DRM = mybir.MatmulPerfMode.DoubleRow

C = 64          # channels
C1 = C + 1      # channels + ones row
C2 = C + 2      # + zero pad (packed layout rows)
CPZ = 99        # packed z partitions (3 groups of 66 rows, /2)
CPU = 99        # packed u partitions
N = 4096        # h*w
NQ = 2048       # query columns per core
NB = 4          # query blocks of 512
QB = 512        # query block width
JT = 128        # j tile width
NJ = 32         # j tiles
NGROUPS = 8
EPS = 1e-5
GSIZE = C // NGROUPS * N

LOG2E = float(np.log2(np.e))
A8 = 0.125 * LOG2E * 4.0
B8 = 60.0 - 0.25

SUMSC = 0.015625   # 2^-6 scaling of the sums row

# cpack layout ([64, CP_COLS] fp32)
CP_G = 0
CP_GAMMA = 8
CP_BETA = 9
CP_GT = 10        # G^T [8, 64] on partitions 0..7
CP_W2T = 74       # (Wp@Wv)^T fp32
CP_BTR = 138      # (Wp@bv + bp) as a row on partition 0
CP_DM = 139       # Dmain[0:64] [64, 64] fp32 (Dmain = [Wq^T;bq] @ Wk)
CP_COLS = 203


def build_bass(stage=5, reps=1, esplit=0.60, chunks=None, npools=3,
               lookahead=3, warmup=8, ering=12, statmode="split"):
    nc = bacc.Bacc("TRN2", target_bir_lowering=False, debug=False, num_devices=8)
    _emit(nc, stage, reps, esplit, chunks, npools, lookahead, warmup, ering,
          statmode)
    nc.compile()
    return nc


def _emit(nc, stage, reps, esplit, chunks_arg, npools, lookahead, warmup,
          ering, statmode):
    xb_d = nc.dram_tensor("xb16", [C1, N], F16, kind="ExternalInput")
    zp_d = nc.dram_tensor("zp8", [CPZ, 2 * N], E4, kind="ExternalInput")
    cp_d = nc.dram_tensor("cpack", [C1, CP_COLS], F32, kind="ExternalInput")
    w2_d = nc.dram_tensor("w2t16", [C, C], F16, kind="ExternalInput")
    out_d = nc.dram_tensor("out", [C, NQ], F32, kind="ExternalOutput")

    chunks = chunks_arg if chunks_arg is not None else [2] * 16
    ERING = ering
    ca = int(round(QB * esplit))

    with tile.TileContext(nc) as tc:
        with (
            tc.tile_pool(name="consts", bufs=2) as consts,
            tc.tile_pool(name="big", bufs=2) as big,
            tc.tile_pool(name="small", bufs=2) as small,
            tc.tile_pool(name="ps_a", bufs=1, space="PSUM") as ps_a,
            tc.tile_pool(name="ps_b", bufs=1, space="PSUM") as ps_b,
            tc.tile_pool(name="ps_d", bufs=1, space="PSUM") as ps_d,
            tc.tile_pool(name="ps_c", bufs=1, space="PSUM") as ps_c,
        ):
          st_pools = [(ps_a, "a1"), (ps_b, "b1"), (ps_d, "d1")][:npools]
          # constants + ACT exp table warm-up
          warm = consts.tile([1, 1], F32, tag="warm")
          nc.vector.memset(warm, 1.0)
          nc.scalar.activation(out=warm, in_=warm, func=AF.Exp, bias=0.0,
                               scale=1.0)
          magic = consts.tile([NGROUPS, 1], mybir.dt.int32, tag="magic")
          nc.vector.memset(magic, 0x5F3759DF)
          c15 = consts.tile([NGROUPS, 1], F32, tag="c15")
          nc.vector.memset(c15, 1.5)
          onesb = consts.tile([1, C], F16, tag="onesb")
          nc.gpsimd.memset(onesb, 1.0)
          # PE warmup fodder: [64, 512] f16, no external deps
          wfod = consts.tile([C, QB], F16, tag="wfod")
          nc.gpsimd.memset(wfod, 0.001)
          cp = consts.tile([C1, CP_COLS], F32, tag="cp")
          w2t = consts.tile([C, C], F16, tag="w2t")
          nc.sync.dma_start(out=cp, in_=cp_d[:, :])
          nc.sync.dma_start(out=w2t, in_=w2_d[:, :])

          for _rep in range(reps):
            # ---- PE warmup spins (fill the DVFS ramp while DMAs land) ----
            if warmup and _rep == 0:
                wps = ps_a.tile([1, QB], F32, tag="a1")
                for _w in range(warmup):
                    nc.tensor.matmul(out=wps, lhsT=wfod[:, 0:1], rhs=wfod,
                                     start=True, stop=True)

            # ---- inputs ----
            x_a = big.tile([C1, N // 2], F16, tag="xa")
            x_b = big.tile([C1, N // 2], F16, tag="xb")
            zp = big.tile([CPZ, 2, N], E4, tag="zp")
            nc.sync.dma_start(out=x_a, in_=xb_d[:, 0:N // 2])
            nc.gpsimd.dma_start(out=x_b, in_=xb_d[:, N // 2:])
            nc.scalar.dma_start(out=zp, in_=zp_d[:, :])

            # ---- GroupNorm stats: 8 partial ops across ACT/DVE/Pool ----
            # s12h col layout (interleaved for tree-add):
            #   0=Sx(a0) 1=Sxx(a0) 2=Sx(a1) 3=Sxx(a1)
            #   4=Sx(b0) 5=Sxx(b0) 6=Sx(b1) 7=Sxx(b1)
            scr_a = big.tile([C, N // 2], F16, tag="scra")
            scr_b = big.tile([C, N // 2], F16, tag="scrb")
            s12 = big.tile([C, 2], F32, tag="s12")
            s12q = big.tile([C, 4], F32, tag="s12q")
            s12h = big.tile([C, 8], F32, tag="s12h")
            # x_a / x_b halves on ACT (Square+accum) and DVE (reduce);
            # col layout 0=Sx(a) 1=Sxx(a) 2=Sx(b) 3=Sxx(b), one tree add.
            nc.scalar.activation(out=scr_a, in_=x_a[0:C, :],
                                 func=AF.Square, accum_out=s12h[:, 1:2])
            nc.vector.reduce_sum(out=s12h[:, 0:1], in_=x_a[0:C, :],
                                 axis=mybir.AxisListType.X)
            nc.scalar.activation(out=scr_b, in_=x_b[0:C, :],
                                 func=AF.Square, accum_out=s12h[:, 3:4])
            nc.vector.reduce_sum(out=s12h[:, 2:3], in_=x_b[0:C, :],
                                 axis=mybir.AxisListType.X)
            nc.vector.tensor_add(out=s12, in0=s12h[:, 0:2],
                                 in1=s12h[:, 2:4])
            gstat = ps_b.tile([NGROUPS, 2], F32, tag="b1")
            nc.tensor.matmul(out=gstat, lhsT=cp[0:C, CP_G:CP_G + NGROUPS],
                             rhs=s12, start=True, stop=True)

            # group mean / E[x^2] -> var -> rstd (Newton on DVE)
            tmv = big.tile([NGROUPS, 2], F32, tag="tmv")
            nc.vector.tensor_scalar_mul(out=tmv, in0=gstat, scalar1=1.0 / GSIZE)
            var = big.tile([NGROUPS, 1], F32, tag="var")
            nc.vector.tensor_mul(out=var, in0=tmv[:, 0:1], in1=tmv[:, 0:1])
            nc.vector.tensor_sub(out=var, in0=tmv[:, 1:2], in1=var)
            veps = big.tile([NGROUPS, 1], F32, tag="veps")
            vh = big.tile([NGROUPS, 1], F32, tag="vh")
            nc.vector.tensor_scalar_add(out=veps, in0=var, scalar1=EPS)
            nc.vector.tensor_scalar_mul(out=vh, in0=veps, scalar1=0.5)
            y_i = big.tile([NGROUPS, 1], mybir.dt.int32, tag="yi")
            nc.vector.tensor_scalar(
                out=y_i, in0=veps.bitcast(mybir.dt.int32), scalar1=1,
                scalar2=None, op0=ALU.arith_shift_right)
            nc.vector.tensor_sub(out=y_i, in0=magic, in1=y_i)
            y_f = y_i.bitcast(F32)
            t_n = big.tile([NGROUPS, 1], F32, tag="tn")
            tgrp = big.tile([NGROUPS, 2], F32, tag="tgrp")
            for _it in range(2):
                nc.vector.tensor_mul(out=t_n, in0=y_f, in1=y_f)
                nc.vector.tensor_mul(out=t_n, in0=t_n, in1=vh)
                nc.vector.scalar_tensor_tensor(
                    out=t_n, in0=t_n, scalar=-1.0, in1=c15,
                    op0=ALU.mult, op1=ALU.add)
                nc.vector.tensor_mul(out=y_f, in0=y_f, in1=t_n)
            nc.vector.tensor_copy(out=tgrp[:, 0:1], in_=y_f)
            nc.vector.tensor_copy(out=tgrp[:, 1:2], in_=tmv[:, 0:1])

            gexp_ps = ps_b.tile([C, 2], F32, tag="b1")
            nc.tensor.matmul(out=gexp_ps, lhsT=cp[0:NGROUPS, CP_GT:CP_GT + C],
                             rhs=tgrp, start=True, stop=True)
            a_s = big.tile([C, 1], F32, tag="a")
            b_s = big.tile([C, 1], F32, tag="b")
            nc.vector.tensor_mul(out=a_s, in0=gexp_ps[:, 0:1],
                                 in1=cp[0:C, CP_GAMMA:CP_GAMMA + 1])
            nc.vector.tensor_mul(out=b_s, in0=gexp_ps[:, 1:2], in1=a_s)
            nc.vector.tensor_sub(out=b_s, in0=cp[0:C, CP_BETA:CP_BETA + 1],
                                 in1=b_s)
            # a66: [a_s; 0; 0] key-side GN scale for the u copy.  Rows 64/65
            # of u would only add per-query constants to the scores (the
            # key-side bias fold), which cancel in softmax -> zero them.
            a66 = big.tile([C2, 1], F32, tag="a66")
            nc.vector.tensor_copy(out=a66[0:C, :], in_=a_s)
            nc.vector.memset(a66[C:C2, :], 0.0)

            # bs1 = [b_s; 1] column (partition 64 = 1.0) for bias rows
            bs1 = big.tile([C1, 1], F32, tag="bs1")
            nc.vector.tensor_copy(out=bs1[0:C, :], in_=b_s)
            nc.vector.memset(bs1[C:C1, :], 1.0)

            # ---- C matrix [65, 66] f16 ----
            # cm[0:64, 0:64] = a_s * Dmain[0:64]  (query-side GN scale fold)
            # cm[64, 0:64]   = Dmain[64] + b_s^T Dmain[0:64]  (query bias row)
            #   computed as bs1^T @ cp[0:65, DM] with the matmul output placed
            #   directly on partition 64 (avoids cross-partition tensor ops)
            # cm[:, 64:66]   = 0 (key-side bias -> softmax-invariant)
            cm = big.tile([C1, C2], F16, tag="cm")
            nc.vector.tensor_scalar_mul(out=cm[0:C, 0:C],
                                        in0=cp[0:C, CP_DM:CP_DM + C],
                                        scalar1=a_s)
            brq_ps = ps_b.tile([C1, C], F32, tag="b1")
            nc.tensor.matmul(out=brq_ps[C:C1, :], lhsT=bs1,
                             rhs=cp[0:C1, CP_DM:CP_DM + C],
                             start=True, stop=True)
            nc.vector.tensor_copy(out=cm[C:C1, 0:C], in_=brq_ps[C:C1, :])
            nc.vector.memset(cm[:, C:C2], 0.0)

            # ---- w22 [66, 66] f16 -> e4m3 packed [33, 2, 66] (row 65 = 0) ----
            w22 = big.tile([C2, C2], F16, tag="w22")
            nc.vector.tensor_scalar_mul(out=w22[0:C, 0:C], in0=w2t,
                                        scalar1=a_s)
            brw_ps = ps_b.tile([C1, C], F32, tag="b1")
            nc.tensor.matmul(out=brw_ps[C:C1, :], lhsT=bs1,
                             rhs=cp[0:C1, CP_W2T:CP_W2T + C], start=True,
                             stop=True)
            nc.vector.memset(w22[C:C2, :], 0.0)
            nc.vector.tensor_copy(out=w22[C:C1, 0:C], in_=brw_ps[C:C1, :])
            nc.gpsimd.memset(w22[0:C, C:C2], 0.0)
            nc.vector.memset(w22[C:C1, C:C1], SUMSC)

            if stage <= 1:
                o1 = big.tile([C, NQ], F32, tag="dbg1")
                nc.vector.tensor_scalar(
                    out=o1, in0=x_a[0:C, 0:NQ], scalar1=a_s, scalar2=b_s,
                    op0=ALU.mult, op1=ALU.add)
                nc.sync.dma_start(out=out_d[:, :], in_=o1)
                return

            # ---- per-block u tiles: r' = C^T x -> e4m3 -> packed ----
            ups = []
            u8cs = []
            for s in range(NB):
                rp = (ps_a if s % 2 == 0 else ps_b).tile(
                    [C2, QB], F32, tag=("a1" if s % 2 == 0 else "b1"))
                nc.tensor.matmul(out=rp, lhsT=cm,
                                 rhs=x_a[:, s * QB:(s + 1) * QB],
                                 start=True, stop=True)
                u8c = big.tile([C2, QB], E4, tag=f"u8c{s}")
                nc.scalar.activation(out=u8c, in_=rp, func=AF.Copy,
                                     scale=a66)
                # residual: u_lo = a66*rp - dequant(u8c), cast to e4m3
                u8r = big.tile([C2, QB], E4, tag=f"u8r{s}")
                nc.vector.scalar_tensor_tensor(
                    out=u8r, in0=rp, scalar=a66, in1=u8c,
                    op0=ALU.mult, op1=ALU.subtract)
                # packed u: groups [u_hi; u_hi; u_lo] pair with z groups
                # [z_hi; z_lo; z_hi] -> dot = z.u - z_lo.u_lo
                up = big.tile([CPU, 2, QB], E4, tag=f"up{s}")
                nc.sync.dma_start(out=up[0:33], in_=u8c)
                nc.gpsimd.dma_start(out=up[33:66], in_=u8c)
                nc.scalar.dma_start(out=up[66:99], in_=u8r)
                ups.append(up)
                u8cs.append(u8c)

            # ---- wg tiles [128, 8, 80-padded] e4m3 via DR matmuls ----
            wg = []
            for g in range(4):
                wg_t = big.tile([JT, 8, 80], E4, tag=f"w{g}")
                wg.append(wg_t)
            for g in range(4):
                # slot stride 128 floats keeps each [128, 66] matmul output
                # inside a single 2KB PSUM bank.  fp16 x * fp16 w22 (not the
                # packed fp8 z) to avoid double-quantizing the PV weights.
                wp_ps = (ps_a if g % 2 == 0 else ps_b).tile(
                    [JT, 8, 128], F32, tag=("a1" if g % 2 == 0 else "b1"))
                for t in range(8):
                    jt = 8 * g + t
                    col = jt * JT
                    xsrc = x_a if col < N // 2 else x_b
                    coff = col if col < N // 2 else col - N // 2
                    nc.tensor.matmul(
                        out=wp_ps[:, t, 0:C2],
                        lhsT=xsrc[:, coff:coff + JT],
                        rhs=w22[0:C1, :],
                        start=True, stop=True)
                nc.scalar.activation(out=wg[g][:, :, 0:C2],
                                     in_=wp_ps[:, :, 0:C2], func=AF.Copy)

            if stage == 2:
                o2 = big.tile([C, NQ], F32, tag="dbg1")
                for s in range(NB):
                    nc.vector.tensor_copy(
                        out=o2[:, s * QB:(s + 1) * QB],
                        in_=u8cs[s][0:C, :])
                nc.sync.dma_start(out=out_d[:, :], in_=o2)
                continue

            # ---- attention ----
            cum = []
            tot = 0
            for cn in chunks:
                tot += cn
                cum.append(tot)
            for b in range(NB if stage >= 5 else 1):
                pv_ps = ps_c.tile([C1, 2, QB], F32, tag="pv")
                pv_main = pv_ps[0:C1, 0, :]
                e_ring = big.tile([JT, ERING, QB], E5, tag="ering")
                e_ri = e_ring.bitcast(U8)
                emitted_j = 0

                def pv_upto(ready_j):
                    nonlocal emitted_j
                    while emitted_j + 2 <= ready_j:
                        jt = emitted_j
                        g, r = jt // 8, jt % 8
                        slot = jt % ERING
                        nc.tensor.matmul(
                            out=pv_main, lhsT=wg[g][:, r:r + 2, 0:C1],
                            rhs=e_ring[:, slot:slot + 2, :],
                            start=(jt == 0), stop=(jt == NJ - 2),
                            skip_group_check=True, perf_mode=DRM)
                        emitted_j += 2

                jt0 = 0
                for ci, cn in enumerate(chunks):
                    pool, tg = st_pools[ci % len(st_pools)]
                    st_ps = pool.tile([JT, cn, QB], F32, tag=tg)
                    for t in range(cn):
                        jt = jt0 + t
                        nc.tensor.matmul(
                            out=st_ps[:, t, :],
                            lhsT=zp[:, :, jt * JT:(jt + 1) * JT],
                            rhs=ups[b], start=True, stop=True,
                            perf_mode=DRM)
                    slot = jt0 % ERING
                    nc.scalar.activation(
                        out=e_ring[:, slot:slot + cn, 0:ca],
                        in_=st_ps[:, 0:cn, 0:ca],
                        func=AF.Exp, scale=0.125)
                    nc.vector.tensor_scalar(
                        out=e_ri[:, slot:slot + cn, ca:QB],
                        in0=st_ps[:, 0:cn, ca:QB],
                        scalar1=A8, scalar2=B8, op0=ALU.mult, op1=ALU.add)
                    if stage >= 4 and ci >= lookahead:
                        pv_upto(cum[ci - lookahead])
                    jt0 += cn
                if stage >= 4:
                    pv_upto(NJ)

                if stage == 3:
                    dbg = small.tile([C, QB], F32, tag="dbg")
                    nc.vector.tensor_copy(out=dbg, in_=st_ps[0:C, 0, :])
                    nc.sync.dma_start(out=out_d[:, 0:QB], in_=dbg)
                    continue
                if stage == 4:
                    dbg = small.tile([C, QB], F32, tag="dbg")
                    nc.vector.tensor_copy(out=dbg, in_=pv_ps[0:C, 0, :])
                    nc.sync.dma_start(out=out_d[:, 0:QB], in_=dbg)
                    dbg2 = small.tile([1, QB], F32, tag="dbg2")
                    nc.vector.tensor_copy(out=dbg2, in_=pv_ps[C:C1, 0, :])
                    nc.sync.dma_start(out=out_d[0:1, QB:2 * QB], in_=dbg2)
                    continue

                # epilogue
                rb = small.tile([1, QB], F16, tag="rb")
                with nc.allow_low_precision(reason="1/sums scaled into fp16"):
                    nc.vector.reciprocal(out=rb, in_=pv_ps[C:C1, 0, :])
                rb_ps = pv_ps[0:C, 1, :]
                nc.tensor.matmul(out=rb_ps, lhsT=onesb, rhs=rb,
                                 start=True, stop=True)
                rbb = small.tile([C, QB], F16, tag="rbb")
                nc.scalar.activation(out=rbb, in_=rb_ps, func=AF.Copy)
                o1_s = small.tile([C, QB], F32, tag="o1")
                nc.vector.scalar_tensor_tensor(
                    out=o1_s, in0=pv_ps[0:C, 0, :], scalar=SUMSC, in1=rbb,
                    op0=ALU.mult, op1=ALU.mult)
                o_s = small.tile([C, QB], F32, tag="o")
                nc.gpsimd.tensor_add(out=o_s, in0=o1_s,
                                     in1=x_a[0:C, b * QB:(b + 1) * QB])
                nc.sync.dma_start(out=out_d[:, b * QB:(b + 1) * QB], in_=o_s)


_NC = {}


def _get_nc(key=None, **kw):
    global _NC
    k = (key, tuple(sorted(kw.items(), key=lambda x: x[0],))) if kw else key
    kk = str(k)
    if kk not in _NC:
        _NC[kk] = build_bass(**kw)
    return _NC[kk]


def make_in_maps(x, gamma, beta, Wq, bq, Wk, bk, Wv, bv, Wp, bp):
    x = np.asarray(x, np.float32)
    b, c, h, w = x.shape
    n = h * w
    xf = x.reshape(b, c, n)
    Wq = np.asarray(Wq, np.float32)
    Wk = np.asarray(Wk, np.float32)
    W2 = np.asarray(Wp, np.float32) @ np.asarray(Wv, np.float32)
    btot0 = np.asarray(Wp, np.float32) @ np.asarray(bv, np.float32) \
        + np.asarray(bp, np.float32)
    # Dmain = [Wq^T; bq^T] @ Wk  [65, 64]
    Wqb = np.concatenate([Wq.T, np.asarray(bq, np.float32)[None, :]], axis=0)
    Dmain = Wqb @ Wk

    cpk = np.zeros((C1, CP_COLS), np.float32)
    cpk[np.arange(C), CP_G + np.arange(C) // (C // NGROUPS)] = 1.0
    cpk[0:C, CP_GAMMA] = np.asarray(gamma, np.float32)
    cpk[0:C, CP_BETA] = np.asarray(beta, np.float32)
    cpk[0:NGROUPS, CP_GT:CP_GT + C] = cpk[0:C, CP_G:CP_G + NGROUPS].T
    cpk[0:C, CP_W2T:CP_W2T + C] = W2.T
    cpk[C, CP_W2T:CP_W2T + C] = btot0
    cpk[:, CP_DM:CP_DM + C] = Dmain

    common = {
        "cpack": cpk,
        "w2t16": np.ascontiguousarray(W2.T.astype(np.float16)),
    }
    in_maps = []
    for core in range(8):
        bi, hi = divmod(core, 2)
        m = dict(common)
        xrot = xf[bi] if hi == 0 else np.concatenate(
            [xf[bi][:, NQ:], xf[bi][:, :NQ]], axis=1)
        xb = np.concatenate(
            [xrot.astype(np.float16), np.ones((1, n), np.float16)], axis=0)
        m["xb16"] = np.ascontiguousarray(xb)
        # packed z, 3 groups of 66 rows: [z_hi; z_lo; z_hi] paired with
        # u groups [u_hi; u_hi; u_lo] so the DR dot = z.u - z_lo.u_lo
        ones = np.ones((1, n), np.float32)
        zeros = np.zeros((1, n), np.float32)
        x_hi8 = xrot.astype(ml_dtypes.float8_e4m3)
        x_lo = (xrot - x_hi8.astype(np.float32)).astype(ml_dtypes.float8_e4m3)
        g1 = np.concatenate([x_hi8.astype(np.float32), ones, zeros], axis=0)
        g2 = np.concatenate([x_lo.astype(np.float32), zeros, zeros], axis=0)
        zfull = np.concatenate([g1, g2, g1], axis=0)  # [198->196? no: 66*3, n]
        zpk = zfull.reshape(CPZ, 2, n).astype(ml_dtypes.float8_e4m3)
        m["zp8"] = np.ascontiguousarray(zpk.reshape(CPZ, 2 * n))
        in_maps.append(m)
    return in_maps


def assemble_out(results, b=4, c=64, h=64, w=64):
    n = h * w
    out = np.empty((b, c, n), np.float32)
    for core in range(8):
        bi, hi = divmod(core, 2)
        r = results[core]["out"]
        if hi == 0:
            out[bi][:, 0:NQ] = r
        else:
            out[bi][:, NQ:] = r
    return out.reshape(b, c, h, w)


def kernel(x, gamma, beta, Wq, bq, Wk, bk, Wv, bv, Wp, bp):
    nc = _get_nc(**globals().get("BUILD_KW", {}))
    in_maps = make_in_maps(x, gamma, beta, Wq, bq, Wk, bk, Wv, bv, Wp, bp)
    res = run_bass_kernel_spmd(nc, in_maps, core_ids=list(range(8)))
    return assemble_out(res.results)


BUILD_KW = {"esplit": 0.54}
